# revision 41
# baseline (speedup 1.0000x reference)
"""Luong 'general' attention kernel for Trainium2 (Bass/Tile), 8-core SPMD.

Computes, for hidden [B,H], encoder_outputs [B,S,H], W [H,H], b [H]:
    q = hidden @ W.T + b                      [B,H]
    energy = einsum('bsh,bh->bs', enc, q)     [B,S]
    attention = softmax(energy, axis=1)       [B,S]
    context = einsum('bsh,bs->bh', enc, att)  [B,H]
returns (context, attention).

Sharding: data-parallel over batch B across 8 NeuronCores (4 batches/core);
W replicated. No cross-core communication.

Per-core dataflow (single pass over enc, which dominates traffic):
  - q phase: W^T (host-pre-transposed) + hidden^T DMA'd in; q computed on PE
    with the bias folded in as a K=1 matmul; q[b] broadcast across the 128
    SBUF partitions with gpsimd.partition_broadcast.
  - per batch b: the 16 enc s-tiles [128s, 1024h] are DMA'd in and stay
    resident in SBUF. Energy e[s] = <enc[s,:], q> via DVE scalar_tensor_tensor
    (fused mul + free-dim sum; note InstTensorTensorReduce crashes this
    device). Softmax: DVE free-dim max -> gpsimd partition all-reduce max ->
    ACT Exp with per-partition bias and fused sum -> gpsimd partition
    all-reduce add -> DVE reciprocal. Context accumulates in PSUM via PE
    matmuls (contraction over s = partition dim), scaled by 1/l on the way
    out. Attention normalized on DVE and emitted via 32x32 stream transposes.

Each batch is processed as two half-batch softmaxes combined exactly at the
end (two-level max/sum, flash-attention style): the first half's context
matmuls overlap the second half's DMA stream, halving the serial tail after
the last DMA. enc is fetched in 1 MB DMAs (two s-tiles per transfer).

Measured (warm-call diff over axon): ~89 us/core for the unified-softmax
version; one-pass traffic floor is 36.2 MB/core (~85 us at the ~425 GB/s
measured per-core HBM rate). The split-softmax tail overlap is worth a
further ~15% in the cost model (220 -> 185 us model time); HW-verified
bit-comparable accuracy (rel err 1.9e-5).
"""

import numpy as np

from concourse import bacc, bass_isa, bass_utils, mybir, tile

B, S, H = 32, 2048, 1024
NCORES = 8
BLOC = B // NCORES          # batches per core
P = 128                     # SBUF partitions
NT = S // P                 # s-tiles per batch
F32 = mybir.dt.float32
ENC_BUFS = 16               # enc double-tile SBUF slots (2 full batches)

# Moving-operand dtype for the context matmuls. float32 is exact but runs the
# PE at 1/4 rate; float32r is single-pass (full rate at N>=256) with slightly
# reduced internal precision. Flipped after measuring accuracy on hardware.
CTX_MM_DTYPE = F32


# Split each batch into two half-batch softmaxes combined exactly at the end
# (flash-attention-style two-level max/sum). Halves the post-last-DMA tail:
# the first half's context matmuls overlap the second half's DMA stream.
SPLIT_SOFTMAX = True
TPH = NT // 2               # s-tiles per half
# Per-half tile indices whose energy dot product runs as gpsimd-mult +
# ACT-fused-reduce instead of DVE scalar_tensor_tensor, pulling DVE (the
# second-busiest engine after DMA) off the critical path. gpsimd cannot run
# TensorScalarPtr (walrus "engine check failed (Pool)") nor free-dim
# tensor_reduce, hence the two-op form.
# DISABLED: the gpsimd-mult + ACT-reduce offload variant hung on hardware
# (560 s timeout, no output) — keep all energy dot products on DVE.
GP_OFFLOAD = ()


def _emit_batch_split(tc, encp, scr, small, ctxps, enc, qbc, ctx_out,
                      att_out, b, mm):
    nc = tc.nc
    X = mybir.AxisListType.X
    Exp = mybir.ActivationFunctionType.Exp
    halves = []
    for h in range(2):
        et = small.tile([P, TPH], F32, name=f"et{h}")
        enc_slices = []
        for t2 in range(TPH // 2):
            enc_t = encp.tile([P, 2, H], F32, name="enc_t")
            s0 = (h * TPH + t2 * 2) * P
            src = enc[b, s0 : s0 + 2 * P, :]
            nc.sync.dma_start(
                out=enc_t[:], in_=src.rearrange("(j p) h -> p j h", p=P)
            )
            for j in range(2):
                t = 2 * t2 + j
                if t in GP_OFFLOAD:
                    prod = scr.tile([P, H], F32, name="prodg", bufs=1)
                    nc.gpsimd.tensor_tensor(
                        prod[:], enc_t[:, j, :], qbc[b][:], mm.mult
                    )
                    nc.scalar.activation(
                        out=prod[:], in_=prod[:],
                        func=mybir.ActivationFunctionType.Copy,
                        accum_out=et[:, t : t + 1],
                    )
                else:
                    prod = scr.tile([P, H], F32, name="prod")
                    nc.vector.scalar_tensor_tensor(
                        out=prod[:], in0=enc_t[:, j, :], scalar=1.0,
                        in1=qbc[b][:], op0=mm.mult, op1=mm.mult,
                        accum_out=et[:, t : t + 1],
                    )
                enc_slices.append(enc_t[:, j, :])

        rowmax = small.tile([P, 1], F32, name=f"rowmax{h}")
        nc.vector.tensor_reduce(out=rowmax[:], in_=et[:], axis=X, op=mm.max)
        mh = small.tile([P, 1], F32, name=f"mh{h}")
        nc.gpsimd.partition_all_reduce(
            mh[:], rowmax[:], channels=P, reduce_op=bass_isa.ReduceOp.max
        )
        negm = small.tile([P, 1], F32, name=f"negm{h}")
        nc.scalar.mul(out=negm[:], in_=mh[:], mul=-1.0)
        ph = small.tile([P, 32], F32, name=f"ph{h}")
        nc.vector.memset(ph[:, TPH:32], 0.0)
        lrow = small.tile([P, 1], F32, name=f"lrow{h}")
        nc.scalar.activation(
            out=ph[:, 0:TPH], in_=et[:], func=Exp, bias=negm[:],
            accum_out=lrow[:],
        )
        lh = small.tile([P, 1], F32, name=f"lh{h}")
        nc.gpsimd.partition_all_reduce(
            lh[:], lrow[:], channels=P, reduce_op=bass_isa.ReduceOp.add
        )
        cps = [
            ctxps.tile([1, 512], F32, name=f"cps{h}{half}", bufs=1)
            for half in range(2)
        ]
        for t in range(TPH):
            for half in range(2):
                cols = slice(half * 512, (half + 1) * 512)
                nc.tensor.matmul(
                    cps[half][:], lhsT=ph[:, t : t + 1],
                    rhs=enc_slices[t][:, cols],
                    start=(t == 0), stop=(t == TPH - 1),
                )
        halves.append((mh, lh, ph, cps))

    (mA, lA, pA, cpsA), (mB, lB, pB, cpsB) = halves
    # exact two-level softmax combine: m = max(mA,mB); w_h = exp(m_h - m);
    # l = wA*lA + wB*lB; ctx = (wA*ctxA + wB*ctxB)/l; att_h = p_h * w_h / l
    mx = small.tile([P, 1], F32, name="mx")
    nc.vector.tensor_tensor(mx[:], mA[:], mB[:], mm.max)
    negmx = small.tile([P, 1], F32, name="negmx")
    nc.scalar.mul(out=negmx[:], in_=mx[:], mul=-1.0)
    wA = small.tile([P, 1], F32, name="wA")
    nc.scalar.activation(out=wA[:], in_=mA[:], func=Exp, bias=negmx[:])
    wB = small.tile([P, 1], F32, name="wB")
    nc.scalar.activation(out=wB[:], in_=mB[:], func=Exp, bias=negmx[:])
    lwA = small.tile([P, 1], F32, name="lwA")
    nc.vector.tensor_tensor(lwA[:], lA[:], wA[:], mm.mult)
    ltot = small.tile([P, 1], F32, name="ltot")
    nc.vector.scalar_tensor_tensor(
        out=ltot[:], in0=lB[:], scalar=1.0, in1=wB[:],
        op0=mm.mult, op1=mm.mult,
    )
    nc.vector.tensor_tensor(ltot[:], ltot[:], lwA[:], mm.add)
    linv = small.tile([P, 1], F32, name="linv")
    nc.vector.reciprocal(linv[:], ltot[:])
    wlA = small.tile([P, 1], F32, name="wlA")
    nc.vector.tensor_tensor(wlA[:], wA[:], linv[:], mm.mult)
    wlB = small.tile([P, 1], F32, name="wlB")
    nc.vector.tensor_tensor(wlB[:], wB[:], linv[:], mm.mult)

    # context: PSUM -> SBUF via ACT with the half-weight folded into scale,
    # then the B-half scaled-add fused into one DVE scalar_tensor_tensor.
    # (Do NOT put the add on gpsimd: mixing plain Pool ops with the extended
    # attn-lib ops makes Bacc insert a ucode library reload per switch —
    # 8 reloads/kernel observed.)
    ctA = small.tile([1, H], F32, name="ctA")
    ctx_sb = small.tile([1, H], F32, name="ctx_sb")
    for half in range(2):
        cols = slice(half * 512, (half + 1) * 512)
        nc.scalar.activation(
            out=ctA[:, cols], in_=cpsA[half][:],
            func=mybir.ActivationFunctionType.Copy, scale=wlA[0:1, :],
        )
        nc.vector.scalar_tensor_tensor(
            out=ctx_sb[:, cols], in0=cpsB[half][:], scalar=wlB[0:1, :],
            in1=ctA[:, cols], op0=mm.mult, op1=mm.add,
        )
    nc.sync.dma_start(out=ctx_out[b : b + 1, :], in_=ctx_sb[:])

    att_v = att_out[b].rearrange("(t s) -> t s", s=P)
    for h, (ph, wl) in enumerate(((pA, wlA), (pB, wlB))):
        pn = small.tile([P, 32], F32, name=f"pn{h}")
        nc.vector.tensor_scalar_mul(out=pn[:], in0=ph[:], scalar1=wl[:])
        at = small.tile([32, P], F32, name=f"at{h}")
        for i in range(4):
            nc.vector.transpose(
                out=at[0:32, i * 32 : (i + 1) * 32],
                in_=pn[i * 32 : (i + 1) * 32, 0:32],
            )
        nc.sync.dma_start(
            out=att_v[h * TPH : (h + 1) * TPH, :], in_=at[0:TPH, :]
        )


def _emit(tc, enc, hT, wT, wb, ctx_out, att_out, repeat=1, ablate=()):
    nc = tc.nc
    mm = mybir.AluOpType

    with (
        tc.tile_pool(name="persist", bufs=1) as persist,
        tc.tile_pool(name="scr", bufs=3) as scr,
        tc.tile_pool(name="small", bufs=2) as small,
        tc.tile_pool(name="ctxps", bufs=2, space="PSUM") as ctxps,
    ):
      for _rep in range(repeat):
        # Load the gpsimd ucode library (partition_broadcast/all_reduce live
        # in 'attn') up front so the ucode DMA overlaps the q-phase weight
        # loads instead of gating the first broadcast in the prefix.
        from concourse import library_config

        nc.gpsimd.load_library(library_config.attn)

        # ---- q phase: q = hidden @ W.T + b, then broadcast per batch ----
        # (own pool scope so the W tiles' SBUF is released to the enc pool)
        qpool_cm = tc.tile_pool(name="qpool", bufs=1)
        qps_cm = tc.tile_pool(name="qps", bufs=1, space="PSUM")
        qpool = qpool_cm.__enter__()
        qps = qps_cm.__enter__()
        wt_tiles = []
        ht_tiles = []
        for k in range(8):
            wt_k = qpool.tile([P, H], F32, name=f"wt{k}")
            nc.sync.dma_start(out=wt_k[:], in_=wT[k * P : (k + 1) * P, :])
            wt_tiles.append(wt_k)
            ht_k = qpool.tile([P, BLOC], F32, name=f"ht{k}")
            nc.sync.dma_start(out=ht_k[:], in_=hT[k * P : (k + 1) * P, :])
            ht_tiles.append(ht_k)
        wb_sb = qpool.tile([1, H], F32, name="wb_sb")
        nc.sync.dma_start(out=wb_sb[:], in_=wb[:])
        ones = qpool.tile([1, BLOC], F32, name="ones")
        nc.vector.memset(ones[:], 1.0)


        q_sb = persist.tile([BLOC, H], F32, name="q_sb")
        for half in range(2):
            cols = slice(half * 512, (half + 1) * 512)
            q_ps = qps.tile([BLOC, 512], F32, name=f"q_ps{half}")
            for k in range(8):
                nc.tensor.matmul(
                    q_ps[:], lhsT=ht_tiles[k][:], rhs=wt_tiles[k][:, cols],
                    start=(k == 0), stop=False,
                )
            nc.tensor.matmul(
                q_ps[:], lhsT=ones[:], rhs=wb_sb[:, cols], start=False, stop=True
            )
            nc.scalar.copy(out=q_sb[:, cols], in_=q_ps[:])

        qbc = []
        for b in range(BLOC):
            # engines can't start at arbitrary partitions; DMA the row to
            # partition 0 first, then broadcast across all 128 partitions
            q_row = persist.tile([1, H], F32, name=f"qrow{b}")
            nc.sync.dma_start(out=q_row[:], in_=q_sb[b : b + 1, :])
            qb = persist.tile([P, H], F32, name=f"qbc{b}")
            nc.gpsimd.partition_broadcast(qb[:], q_row[:], channels=P)
            qbc.append(qb)

        qps_cm.__exit__(None, None, None)
        qpool_cm.__exit__(None, None, None)

        # ---- main loop: one batch at a time, enc tiles resident in SBUF ----
        encp_cm = tc.tile_pool(name="encp", bufs=ENC_BUFS)
        encp = encp_cm.__enter__()
        for b in range(BLOC):
          if SPLIT_SOFTMAX:
            _emit_batch_split(tc, encp, scr, small, ctxps, enc, qbc,
                              ctx_out, att_out, b, mm)
            continue
          else:
            enc_tiles = []
            et = small.tile([P, NT], F32, name="et")
            # 1 MB DMAs: two s-tiles per transfer (partition p reads rows
            # {s0+p, s0+128+p}, i.e. 2x4KB contiguous descriptors each)
            for t2 in range(NT // 2):
                enc_t = encp.tile([P, 2, H], F32, name="enc_t")
                src = enc[b, t2 * 2 * P : (t2 + 1) * 2 * P, :]
                nc.sync.dma_start(
                    out=enc_t[:], in_=src.rearrange("(j p) h -> p j h", p=P)
                )
                for j in range(2):
                    t = 2 * t2 + j
                    # e[s] = sum_h enc[s,h]*q[h]: fused mul + free-dim sum on
                    # DVE (InstTensorTensorReduce crashes the device;
                    # TensorScalarPtr with is_scalar_tensor_tensor is the
                    # working equivalent)
                    if "energy" not in ablate:
                        prod = scr.tile([P, H], F32, name="prod")
                        nc.vector.scalar_tensor_tensor(
                            out=prod[:], in0=enc_t[:, j, :], scalar=1.0,
                            in1=qbc[b][:], op0=mm.mult, op1=mm.mult,
                            accum_out=et[:, t : t + 1],
                        )
                    else:
                        nc.vector.memset(et[:, t : t + 1], 1.0)
                    enc_tiles.append(enc_t[:, j, :])

            # softmax statistics over all S positions of batch b
            rowmax = small.tile([P, 1], F32, name="rowmax")
            nc.vector.tensor_reduce(
                out=rowmax[:], in_=et[:], axis=mybir.AxisListType.X, op=mm.max
            )
            mb = small.tile([P, 1], F32, name="mb")
            if "par" not in ablate:
                nc.gpsimd.partition_all_reduce(
                    mb[:], rowmax[:], channels=P,
                    reduce_op=bass_isa.ReduceOp.max,
                )
            else:
                nc.vector.tensor_copy(out=mb[:], in_=rowmax[:])
            negm = small.tile([P, 1], F32, name="negm")
            nc.scalar.mul(out=negm[:], in_=mb[:], mul=-1.0)

            pt = small.tile([P, 32], F32, name="pt")
            nc.vector.memset(pt[:, NT:32], 0.0)
            lrow = small.tile([P, 1], F32, name="lrow")
            nc.scalar.activation(
                out=pt[:, 0:NT], in_=et[:],
                func=mybir.ActivationFunctionType.Exp,
                bias=negm[:], accum_out=lrow[:],
            )
            lb = small.tile([P, 1], F32, name="lb")
            if "par" not in ablate:
                nc.gpsimd.partition_all_reduce(
                    lb[:], lrow[:], channels=P, reduce_op=bass_isa.ReduceOp.add
                )
            else:
                nc.vector.tensor_copy(out=lb[:], in_=lrow[:])
            linv = small.tile([P, 1], F32, name="linv")
            nc.vector.reciprocal(linv[:], lb[:])

            # context[b,:] = sum_s exp(e_s - m) * enc[s,:], scaled by 1/l
            cps = [
                ctxps.tile([1, 512], F32, name=f"cps{half}") for half in range(2)
            ]
            for t in range(NT):
                if "ctx" in ablate:
                    break
                for half in range(2):
                    cols = slice(half * 512, (half + 1) * 512)
                    lhs = pt[:, t : t + 1]
                    rhs = enc_tiles[t][:, cols]
                    if CTX_MM_DTYPE is not F32:
                        lhs = lhs.bitcast(CTX_MM_DTYPE)
                        rhs = rhs.bitcast(CTX_MM_DTYPE)
                    nc.tensor.matmul(
                        cps[half][:], lhsT=lhs, rhs=rhs,
                        start=(t == 0), stop=(t == NT - 1),
                    )
            if "ctx" not in ablate:
                ctx_sb = small.tile([1, H], F32, name="ctx_sb")
                for half in range(2):
                    cols = slice(half * 512, (half + 1) * 512)
                    nc.scalar.activation(
                        out=ctx_sb[:, cols], in_=cps[half][:],
                        func=mybir.ActivationFunctionType.Copy,
                        scale=linv[0:1, :],
                    )
                nc.sync.dma_start(out=ctx_out[b : b + 1, :], in_=ctx_sb[:])

            # attention[b,:] = exp(e - m) / l, transposed out via 32x32 blocks
            pn = small.tile([P, 32], F32, name="pn")
            nc.vector.tensor_scalar_mul(out=pn[:], in0=pt[:], scalar1=linv[:])
            at = small.tile([32, P], F32, name="at")
            for i in range(4):
                nc.vector.transpose(
                    out=at[0:32, i * 32 : (i + 1) * 32],
                    in_=pn[i * 32 : (i + 1) * 32, 0:32],
                )
            nc.sync.dma_start(
                out=att_out[b].rearrange("(t s) -> t s", s=P), in_=at[0:NT, :]
            )
        encp_cm.__exit__(None, None, None)


def build(repeat=1, ablate=()):
    # Bacc (not raw Bass): its compile() splits multi-waits off matmuls,
    # inserts gpsimd ucode library loads, ACT table loads, and lowers
    # extended-inst ISA bytes — all required by the NEFF compiler.
    nc = bacc.Bacc(
        "TRN2", target_bir_lowering=False, debug=False, num_devices=NCORES
    )
    enc = nc.dram_tensor("enc", [BLOC, S, H], F32, kind="ExternalInput").ap()
    hT = nc.dram_tensor("hT", [H, BLOC], F32, kind="ExternalInput").ap()
    wT = nc.dram_tensor("wT", [H, H], F32, kind="ExternalInput").ap()
    wb = nc.dram_tensor("wb", [1, H], F32, kind="ExternalInput").ap()
    ctx_out = nc.dram_tensor("ctx", [BLOC, H], F32, kind="ExternalOutput").ap()
    att_out = nc.dram_tensor("att", [BLOC, S], F32, kind="ExternalOutput").ap()
    with tile.TileContext(nc) as tc:
        _emit(tc, enc, hT, wT, wb, ctx_out, att_out, repeat=repeat,
              ablate=ablate)
    nc.compile()
    return nc


_nc_cache = None


def _get_nc():
    global _nc_cache
    if _nc_cache is None:
        _nc_cache = build()
    return _nc_cache


def make_in_maps(hidden, encoder_outputs, W_weight, W_bias):
    hidden = np.asarray(hidden, dtype=np.float32)
    encoder_outputs = np.asarray(encoder_outputs, dtype=np.float32)
    wT = np.ascontiguousarray(np.asarray(W_weight, dtype=np.float32).T)
    wb = np.ascontiguousarray(np.asarray(W_bias, dtype=np.float32).reshape(1, H))
    in_maps = []
    for c in range(NCORES):
        sl = slice(c * BLOC, (c + 1) * BLOC)
        in_maps.append(
            {
                "enc": np.ascontiguousarray(encoder_outputs[sl]),
                "hT": np.ascontiguousarray(hidden[sl].T),
                "wT": wT,
                "wb": wb,
            }
        )
    return in_maps


def kernel(hidden, encoder_outputs, W_weight, W_bias):
    nc = _get_nc()
    in_maps = make_in_maps(hidden, encoder_outputs, W_weight, W_bias)
    res = bass_utils.run_bass_kernel_spmd(nc, in_maps, list(range(NCORES))).results
    context = np.concatenate([res[c]["ctx"] for c in range(NCORES)], axis=0)
    attention = np.concatenate([res[c]["att"] for c in range(NCORES)], axis=0)
    return context, attention
